# revision 16
# baseline (speedup 1.0000x reference)
"""Trainium2 Bass kernel for causal MultiHeadAttention + residual + LayerNorm.

Problem: nn_MultiHeadAttention_88124138979383
  B=2, L=2048, D=1024, H=16, DH=64, causal mask, out-proj, residual, LN.

Sharding (8 cores): core c = (batch b=c//4, head-group g=c%4, 4 heads each).
Each core projects Q^T/K^T (head-dim on partitions) and V (natural layout)
for its batch+heads, runs causal attention in scores-transposed layout
(softmax reduction via a ones-column appended to V inside the P@V matmul,
no max-subtraction), normalizes via a GPSIMD partition-broadcast of the
reciprocal denominator, then an 8-core AllToAll exchanges head-shards for
sequence-shards per q-block; after AllToAll #qb, core c has the full
16-head A^T for queries [512qb+64c : +64] of BOTH batches and computes the
output projection, residual (query rows + bo pre-added on host) and
LayerNorm for those 128 rows.

v2 schedule: the whole rep is one software-pipelined PE stream. The
attention inner loop (score matmuls -> merged 2-head exp on ACT -> PV
accumulate, with the PV lagging 2 k-tiles) is the backbone; the Q/K/V
projection matmul groups of the NEXT q-block and the out-projection groups
of the PREVIOUS q-block are interleaved into it as fillers, so the PE
never drains (keeps the tensor engine at full DVFS clock and gives the
ACT engine slack to keep up). Steady state per rep:
  attn(0)[fill: proj(1), outproj(3 of prev rep)]
  attn(1)[fill: proj(2), outproj(0)]
  attn(2)[fill: proj(3), outproj(1)]
  attn(3)[fill: proj(0 of next rep), outproj(2)]
Both heads of a pair share one [128, 2, 512] PSUM score tile (2 banks) so
a single ACT instruction exponentiates both. PSUM: 2x s2 (4 banks) +
1x at2 (2 banks) + 2 shared proj/outproj banks = 8.

Causal masking is done on the PE: a small extra matmul accumulates -240
into the masked upper-triangle band of the diagonal score tiles, so
exp() drives masked entries to 0. LayerNorm's 1/sqrt(var+eps) is computed
as exp(-0.5*log(var+eps)) so the scalar engine stays on the exp/log table
set (one load, no thrash). All matmuls bf16 (fp32 PSUM accumulate).
"""
import os
import sys

for _p in ("/opt/trn_rl_repo", os.path.join(os.path.expanduser("~"), ".axon_site", "_ro", "trn_rl_repo")):
    if os.path.isdir(_p) and _p not in sys.path:
        sys.path.insert(0, _p)

import numpy as np
import ml_dtypes

import concourse.bass as bass
import concourse.tile as tile
from concourse import bacc, mybir
from concourse.hw_specs import get_activation_tables as _real_gat


def _gat_prefer_combined(arch):
    """Table-set view where Exp/Ln are only satisfiable by the combined
    natural_log_exp_and_others set, so the ACT table is loaded once instead
    of thrashing between exp- and ln-anchored sets on every LayerNorm.
    Entries/order (= act_func_set_id) are unchanged; runtime tables are the
    real ones, so numerics are unaffected."""
    full = _real_gat(arch)
    exp_f = mybir.ActivationFunctionType.Exp
    ln_f = mybir.ActivationFunctionType.Ln
    combined = full.get("natural_log_exp_and_others")
    if not combined or exp_f not in combined or ln_f not in combined:
        return full
    out = {}
    for name, funcs in full.items():
        f = set(funcs)
        if name != "natural_log_exp_and_others":
            f.discard(exp_f)
            f.discard(ln_f)
        out[name] = f
    return out


bacc.get_activation_tables = _gat_prefer_combined

BF16 = ml_dtypes.bfloat16
F32 = mybir.dt.float32
BF = mybir.dt.bfloat16

B, L, D = 2, 2048, 1024
H, DH = 16, 64
NCORES = 8
HL = 4                 # heads per core
NPAIR = 2              # head pairs per core
MBS = 512              # m-block size for projections
NMB = L // MBS         # 4
QBS = 512              # q-block size for attention
NQB = L // QBS         # 4
KTS = 128              # k-tile size
NKT = L // KTS         # 16
MS = L // NCORES       # 256: per-core row count for out-proj/LN (both batches)
LN_EPS = 1e-5
SCALE = 1.0 / 8.0      # 1/sqrt(DH)
NEG = -240.0           # causal mask additive constant (exp(NEG/8) ~= 0)
PVLAG = 3              # PV lags the score/exp pipeline by this many k-tiles


def build_nc(reps: int = 1, phases: str = 'full', with_biases: bool = False,
             ln_trivial: bool = True):
    nc = bacc.Bacc("TRN2", target_bir_lowering=False, debug=False, num_devices=NCORES)
    qT = nc.dram_tensor("qT", [D, L], BF, kind="ExternalInput")
    kT = nc.dram_tensor("kT", [D, L], BF, kind="ExternalInput")
    vT = nc.dram_tensor("vT", [D, L], BF, kind="ExternalInput")
    wq = nc.dram_tensor("wq", [D, HL * DH], BF, kind="ExternalInput")
    wk = nc.dram_tensor("wk", [D, HL * DH], BF, kind="ExternalInput")
    wv = nc.dram_tensor("wv", [D, HL * DH], BF, kind="ExternalInput")
    wo = nc.dram_tensor("wo", [D, D], BF, kind="ExternalInput")
    bq = nc.dram_tensor("bq", [HL * DH, 1], F32, kind="ExternalInput")
    bk = nc.dram_tensor("bk", [HL * DH, 1], F32, kind="ExternalInput")
    bv = nc.dram_tensor("bv", [DH, HL], F32, kind="ExternalInput")
    qresbo = nc.dram_tensor("qresbo", [2 * MS, D], F32, kind="ExternalInput")
    gamma = nc.dram_tensor("gamma", [1, D], F32, kind="ExternalInput")
    beta = nc.dram_tensor("beta", [1, D], F32, kind="ExternalInput")
    y = nc.dram_tensor("y", [2 * MS, D], F32, kind="ExternalOutput")

    with tile.TileContext(nc) as tc:
        with (
            tc.tile_pool(name="consts", bufs=1) as consts,
            tc.tile_pool(name="persist", bufs=1) as persist,
            tc.tile_pool(name="xin", bufs=2) as xin,
            tc.tile_pool(name="es", bufs=6) as espool,
            tc.tile_pool(name="work", bufs=2) as work,
            tc.tile_pool(name="ps_mm", bufs=2, space="PSUM") as ps_mm,
            tc.tile_pool(name="ps_s", bufs=2, space="PSUM") as ps_s,
            tc.tile_pool(name="ps_acc", bufs=1, space="PSUM") as ps_acc,
            tc.tile_pool(name="dram", bufs=1, space="DRAM") as dram,
        ):
            # ---- constants / weights ----
            wq_sb = consts.tile([128, 8, HL * DH], BF, tag="wq")
            wk_sb = consts.tile([128, 8, HL * DH], BF, tag="wk")
            wv_sb = consts.tile([128, 8, HL * DH], BF, tag="wv")
            for hf in range(2):
                nc.sync.dma_start(out=wq_sb[:, 4 * hf:4 * hf + 4, :],
                                  in_=wq[512 * hf:512 * hf + 512, :].rearrange("(t p) n -> p t n", p=128))

            def load_wkv():
                nc.sync.dma_start(out=wk_sb, in_=wk.rearrange("(t p) n -> p t n", p=128))
                nc.sync.dma_start(out=wv_sb, in_=wv.rearrange("(t p) n -> p t n", p=128))
            wo_sb = consts.tile([128, 8, D], BF, tag="wo")
            bq_sb = consts.tile([128, NPAIR], F32, tag="bq")
            bk_sb = consts.tile([128, NPAIR], F32, tag="bk")
            nc.sync.dma_start(out=bq_sb, in_=bq.rearrange("(t p) o -> p (t o)", p=128))
            nc.sync.dma_start(out=bk_sb, in_=bk.rearrange("(t p) o -> p (t o)", p=128))
            bv_sb = consts.tile([DH, HL], F32, tag="bv")
            nc.sync.dma_start(out=bv_sb, in_=bv[:, :])
            eps_sb = consts.tile([128, 1], F32, tag="eps")
            nc.gpsimd.memset(eps_sb, LN_EPS)
            # causal-mask matmul constants: negT[r, c] = NEG where r < c else 0
            # (strictly upper); ident = I. PE adds into the diagonal band of a
            # score tile: s[p, q'] += sum_k negT[k, p]*I[k, q'] = NEG where q'<p.
            negT_sb = consts.tile([128, KTS], BF, tag="negT")
            nc.gpsimd.memset(negT_sb, NEG)
            nc.gpsimd.affine_select(
                out=negT_sb, in_=negT_sb,
                compare_op=mybir.AluOpType.is_ge, fill=0.0,
                base=-1, channel_multiplier=-1, pattern=[[1, KTS]])
            ident_sb = consts.tile([128, KTS], BF, tag="ident")
            nc.gpsimd.memset(ident_sb, 1.0)
            nc.gpsimd.affine_select(
                out=ident_sb, in_=ident_sb,
                compare_op=mybir.AluOpType.is_ge, fill=0.0,
                base=0, channel_multiplier=-1, pattern=[[1, KTS]])
            nc.gpsimd.affine_select(
                out=ident_sb, in_=ident_sb,
                compare_op=mybir.AluOpType.is_ge, fill=0.0,
                base=0, channel_multiplier=1, pattern=[[-1, KTS]])

            gam_sb = consts.tile([128, D], F32, tag="gam")
            bet_sb = consts.tile([128, D], F32, tag="bet")
            qres_sb4 = consts.tile([128, NQB, D], F32, tag="qres4")

            # ---- persistent activations ----
            qT_sb = [persist.tile([128, L], BF, tag=f"qT{p}", name=f"qT_sb{p}") for p in range(NPAIR)]
            kT_sb = [persist.tile([128, L], BF, tag=f"kT{p}", name=f"kT_sb{p}") for p in range(NPAIR)]
            # V in natural [seq, d] layout, 128 cols per head: col 0 = ones
            # (softmax denominator lands at PSUM partition 0, where GPSIMD
            # partition_broadcast can read it), cols 1:64 zero pad, 64:128 = V
            # (A rows land at partitions 64:128; DVE partition-shifts them to
            # 0:64 during the normalize multiply)
            v128_sb = persist.tile([128, NKT, HL * 128], BF, tag="v128")
            nc.gpsimd.memset(v128_sb, 0.0)
            nc.gpsimd.memset(
                v128_sb.rearrange("p kt (h x) -> p kt h x", x=128)[:, :, :, 0:1], 1.0)
            # normalized attention output A^T: [DH, head, L]
            a4_sb = persist.tile([DH, HL, L], BF, tag="a4", name="a4_sb")
            # gathered A^T after per-qb A2A: [part, qb, ctile, batch, m]
            ob2_sb = persist.tile([128, NQB, 8, 2, DH], BF, tag="ob2", name="ob2_sb")

            in_bq = [dram.tile([L, DH], BF, name=f"in_bq{i}") for i in range(NQB)]
            out_bq = [dram.tile([L, DH], BF, name=f"out_bq{i}") for i in range(NQB)]

            xin_tiles = {}

            def issue_loads(mb, split_in=False):
                m0 = mb * MBS
                key = mb
                xq = xin.tile([128, 8, MBS], BF, tag="xq", name=f"xq{mb}")
                xk = xin.tile([128, 8, MBS], BF, tag="xk", name=f"xk{mb}")
                xv = xin.tile([128, 8, MBS], BF, tag="xv", name=f"xv{mb}")
                xin_tiles[key] = (xq, xk, xv)
                if split_in:
                    # halve the first transfers so the first matmuls start sooner;
                    # wk/wv load after xq (needed only once Q's matmuls are running)
                    for hf in range(2):
                        nc.sync.dma_start(
                            out=xq[:, 4 * hf:4 * hf + 4, :],
                            in_=qT[512 * hf:512 * hf + 512, m0:m0 + MBS].rearrange(
                                "(t p) m -> p t m", p=128))
                    load_wkv()
                    for src_t, dst in ((kT, xk), (vT, xv)):
                        for hf in range(2):
                            nc.sync.dma_start(
                                out=dst[:, 4 * hf:4 * hf + 4, :],
                                in_=src_t[512 * hf:512 * hf + 512, m0:m0 + MBS].rearrange(
                                    "(t p) m -> p t m", p=128))
                else:
                    nc.sync.dma_start(out=xq, in_=qT[:, m0:m0 + MBS].rearrange("(t p) m -> p t m", p=128))
                    nc.sync.dma_start(out=xk, in_=kT[:, m0:m0 + MBS].rearrange("(t p) m -> p t m", p=128))
                    nc.sync.dma_start(out=xv, in_=vT[:, m0:m0 + MBS].rearrange("(t p) m -> p t m", p=128))

            def proj_groups(mb):
                """8 filler closures: Q-pair0, Q-pair1, K-pair0, K-pair1, 4x V."""
                m0 = mb * MBS
                xq, xk, xv = xin_tiles[mb]

                def g_qk(p, w_sb, x_t, dst, b_sb):
                    def run():
                        ps = ps_mm.tile([128, MBS], F32, tag="mm", name="ps_qk")
                        for t in range(8):
                            nc.tensor.matmul(ps[:], w_sb[:, t, 128 * p:128 * p + 128],
                                             x_t[:, t, :], start=(t == 0), stop=(t == 7))
                        if with_biases:
                            nc.vector.tensor_scalar_add(dst[:, m0:m0 + MBS], ps[:], b_sb[:, p:p + 1])
                        else:
                            nc.vector.tensor_copy(dst[:, m0:m0 + MBS], ps[:])
                    return run

                def g_v(ms):
                    def run():
                        mt = mb * (MBS // 128) + ms
                        psv = ps_mm.tile([128, HL * DH], F32, tag="mm", name="ps_v")
                        for t in range(8):
                            nc.tensor.matmul(psv[:], xv[:, t, 128 * ms:128 * ms + 128],
                                             wv_sb[:, t, :], start=(t == 0), stop=(t == 7))
                        dst = v128_sb[:, mt, :].rearrange("p (h x) -> p h x", x=128)[:, :, 64:128]
                        nc.vector.tensor_copy(dst, psv[:].rearrange("p (h x) -> p h x", x=DH))
                    return run

                gs = [g_qk(0, wq_sb, xq, qT_sb[0], bq_sb), g_qk(1, wq_sb, xq, qT_sb[1], bq_sb),
                      g_qk(0, wk_sb, xk, kT_sb[0], bk_sb), g_qk(1, wk_sb, xk, kT_sb[1], bk_sb)]
                gs += [g_v(ms) for ms in range(MBS // 128)]

                def last_wrap(fn):
                    def run():
                        fn()
                        xin_tiles.pop(mb, None)
                    return run
                gs[-1] = last_wrap(gs[-1])
                return gs

            def outproj_fillers(qb):
                """4 filler closures: gather+mm-lo, mm-hi, stats, LN+store."""
                x_sb = work.tile([128, D], F32, tag="x", name=f"x_sb{qb}")
                stats = work.tile([128, 2, 6], F32, tag="stats", name=f"stats{qb}")
                mv = work.tile([128, 2], F32, tag="mv", name=f"mv{qb}")

                def g_mm(nb):
                    def run():
                        if nb == 0:
                            # gather waits on AllToAll #qb; placed mid-block so
                            # the collective is done and the DMA queue never
                            # head-of-line blocks
                            for b_ in range(2):
                                nc.sync.dma_start(
                                    out=ob2_sb[:, qb, :, b_, :],
                                    in_=out_bq[qb].rearrange("(b t p) m -> p t b m", b=2, t=8, p=128)[:, :, b_])
                        o_ps = ps_mm.tile([128, 512], F32, tag="mm", name="o_ps")
                        for t in range(8):
                            nc.tensor.matmul(o_ps[:],
                                             ob2_sb[:, qb, t, :, :],
                                             wo_sb[:, t, 512 * nb:512 * nb + 512],
                                             start=(t == 0), stop=(t == 7))
                        nc.vector.tensor_add(x_sb[:, 512 * nb:512 * nb + 512], o_ps[:],
                                             qres_sb4[:, qb, 512 * nb:512 * nb + 512])
                    return run

                def g_stats():
                    nc.vector.bn_stats(out=stats[:, 0, :], in_=x_sb[:, 0:512])
                    nc.vector.bn_stats(out=stats[:, 1, :], in_=x_sb[:, 512:1024])
                    nc.vector.bn_aggr(out=mv[:], in_=stats[:])

                def g_ln():
                    # rstd = 1/sqrt(var+eps) = exp(-0.5*log(var+eps)): stays
                    # inside the exp/log ACT table set (no table switch)
                    rstd = work.tile([128, 1], F32, tag="rstd")
                    nc.scalar.activation(out=rstd[:], in_=mv[:, 1:2],
                                         func=mybir.ActivationFunctionType.Ln,
                                         bias=eps_sb[:, 0:1], scale=1.0)
                    nc.scalar.activation(out=rstd[:], in_=rstd[:],
                                         func=mybir.ActivationFunctionType.Exp, scale=-0.5)
                    y_sb = work.tile([128, D], F32, tag="y")
                    nc.vector.tensor_scalar(out=y_sb[:], in0=x_sb[:],
                                            scalar1=mv[:, 0:1], scalar2=rstd[:, 0:1],
                                            op0=mybir.AluOpType.subtract,
                                            op1=mybir.AluOpType.mult)
                    if not ln_trivial:
                        nc.vector.scalar_tensor_tensor(out=y_sb[:], in0=y_sb[:], scalar=1.0,
                                                       in1=gam_sb[:],
                                                       op0=mybir.AluOpType.mult,
                                                       op1=mybir.AluOpType.mult)
                        nc.vector.tensor_add(y_sb[:], y_sb[:], bet_sb[:])
                    nc.sync.dma_start(out=y[128 * qb:128 * qb + 128, :], in_=y_sb[:])

                return [g_mm(0), g_mm(1), g_stats, g_ln]

            def pv_emit(p, at2, nkt, kt, off, es2):
                for i in range(2):
                    nc.tensor.matmul(at2[:, i, off:],
                                     v128_sb[:, kt, 128 * (2 * p + i):128 * (2 * p + i) + 128],
                                     es2[:, i, off:], start=(kt == 0), stop=(kt == nkt - 1))

            def normalize(p, at2, qb):
                q0 = qb * QBS
                rec = work.tile([1, 2, QBS], BF, tag="rec")
                with nc.allow_low_precision("bf16 softmax reciprocal is within tolerance"):
                    nc.vector.reciprocal(out=rec[:, :, :], in_=at2[0:1, :, :])
                bc_sb = work.tile([64, 2, QBS], BF, tag="bc_sb")
                nc.gpsimd.partition_broadcast(bc_sb[:, :, :], rec[:, :, :])
                for i in range(2):
                    h = 2 * p + i
                    nc.vector.tensor_mul(a4_sb[:, h, q0:q0 + QBS], at2[64:128, i, :], bc_sb[:, i, :])
                    if with_biases:
                        nc.vector.tensor_scalar_add(a4_sb[:, h, q0:q0 + QBS],
                                                    a4_sb[:, h, q0:q0 + QBS],
                                                    bv_sb[:, h:h + 1])

            def attn_block(qb, fillers, early_fillers=(), filler_min_unit=0,
                           do_a2a=True):
                q0 = qb * QBS
                nkt = 4 * qb + 4
                n_units = 2 * nkt
                slots = {}
                # outproj fillers go in the second half of the block: their
                # AllToAll completed >=2 blocks ago, and their Pool-side
                # residual adds then queue after this block's first
                # partition_broadcast instead of in front of the projection
                # copies (in-order Pool queue)
                eu0 = n_units // 2
                ne = len(early_fillers)
                for fi, f in enumerate(early_fillers):
                    u = eu0 + (fi * (n_units - eu0)) // ne if ne else 0
                    slots.setdefault(min(u, n_units - 1), []).append(f)
                # projection fillers go in the first half (all their Pool
                # copies queue before the first normalize), unless the
                # rep-boundary WAR forces them later
                nf = len(fillers)
                hi = n_units // 2 if filler_min_unit < n_units // 2 else n_units
                span = max(1, hi - filler_min_unit)
                for fi, f in enumerate(fillers):
                    u = filler_min_unit + (fi * span) // nf if nf else 0
                    slots.setdefault(min(u, n_units - 1), []).append(f)
                u = 0
                stripped = phases in ('sc', 'scexp', 'scexpsplit')
                for p in range(NPAIR):
                    at2 = None if stripped else ps_acc.tile([128, 2, QBS], F32, tag="acc", name="at2")
                    pend = []
                    for kt in range(nkt):
                        for f in slots.pop(u, ()):  # fillers keep the PE fed
                            f()
                        k0 = kt * KTS
                        d = kt - 4 * qb
                        diag = d >= 0
                        # causally-valid q-slice (cols < off are fully masked)
                        off = 128 * d if d > 0 else 0
                        s2 = ps_s.tile([128, 2, QBS], F32, tag="s", name="s2")
                        nc.tensor.matmul(s2[:, 0, :], kT_sb[p][0:64, k0:k0 + KTS],
                                         qT_sb[p][0:64, q0:q0 + QBS],
                                         start=True, stop=not diag)
                        nc.tensor.matmul(s2[:, 1, :], kT_sb[p][64:128, k0:k0 + KTS],
                                         qT_sb[p][64:128, q0:q0 + QBS],
                                         start=True, stop=not diag)
                        if diag:  # diagonal-crossing tile: add NEG to masked band
                            b0 = 128 * d
                            for i in range(2):
                                nc.tensor.matmul(s2[:, i, b0:b0 + KTS], negT_sb, ident_sb,
                                                 start=False, stop=True)
                        if phases == 'sc':
                            u += 1
                            continue
                        es2 = espool.tile([128, 2, QBS], BF, tag="es")
                        if phases == 'scexpsplit':
                            for i in range(2):
                                nc.scalar.activation(out=es2[:, i, off:], in_=s2[:, i, off:],
                                                     func=mybir.ActivationFunctionType.Exp,
                                                     scale=SCALE)
                        else:
                            nc.scalar.activation(out=es2[:, :, off:], in_=s2[:, :, off:],
                                                 func=mybir.ActivationFunctionType.Exp,
                                                 scale=SCALE)
                        if phases in ('scexp', 'scexpsplit'):
                            u += 1
                            continue
                        pend.append((kt, off, es2))
                        if len(pend) > PVLAG:
                            pv_emit(p, at2, nkt, *pend.pop(0))
                        u += 1
                    for item in pend:
                        pv_emit(p, at2, nkt, *item)
                    if not stripped:
                        normalize(p, at2, qb)
                # leftover fillers (defensive)
                for us in sorted(slots):
                    for f in slots[us]:
                        f()
                if not do_a2a:
                    return
                # A2A input for this q-block: dest chunk j gets A^T cols
                # [512qb+64j : +64] in [(h p) m] row layout
                for h in range(HL):
                    nc.sync.dma_start(
                        out=in_bq[qb].rearrange("(j h p) m -> p h j m", j=NCORES, h=HL, p=DH)[:, h],
                        in_=a4_sb[:, h, q0:q0 + QBS].rearrange("p (j m) -> p j m", j=NCORES))
                nc.gpsimd.collective_compute(
                    "AllToAll", mybir.AluOpType.bypass,
                    ins=[in_bq[qb].opt()], outs=[out_bq[qb].opt()],
                    replica_groups=[list(range(NCORES))])

            full = phases == 'full'
            for _rep in range(reps):
                first, last = _rep == 0, _rep == reps - 1
                if first:
                    issue_loads(0, split_in=True)
                    issue_loads(1)
                    # E-phase constants; after the xin loads so they don't
                    # delay the projection pipeline
                    nc.sync.dma_start(out=wo_sb, in_=wo.rearrange("(t p) n -> p t n", p=128))
                    nc.sync.dma_start(out=qres_sb4,
                                      in_=qresbo.rearrange("(r p) n -> p r n", p=128))
                    if not ln_trivial:
                        nc.sync.dma_start(out=gam_sb, in_=gamma[:, :].to_broadcast([128, D]))
                        nc.sync.dma_start(out=bet_sb, in_=beta[:, :].to_broadcast([128, D]))
                    for g in proj_groups(0):
                        g()
                if phases == 'proj':
                    for mb in range(NMB):
                        if not (first and mb <= 1):
                            issue_loads(mb)
                        if not (first and mb == 0):
                            for g in proj_groups(mb):
                                g()
                    continue
                # outproj(qb) runs three blocks after its AllToAll (in the
                # second half of attn(qb+3)) so the ~20us collective latency
                # plus gather is hidden with ~40us of margin and the gather
                # DMA never fronts a collective wait
                for qb in range(NQB):
                    fillers, early = [], []
                    if qb == 0:
                        issue_loads(2)
                        fillers += proj_groups(1)
                        if full and not first:
                            early += outproj_fillers(1)
                    elif qb == 1:
                        issue_loads(3)
                        fillers += proj_groups(2)
                        if full and not first:
                            early += outproj_fillers(2)
                    elif qb == 2:
                        if not last:
                            issue_loads(0)
                        fillers += proj_groups(3)
                        if full and not first:
                            early += outproj_fillers(3)
                    else:
                        if full:
                            early += outproj_fillers(0)
                            if last:
                                # tail trim: outproj(1)'s collective finished
                                # during attn(2); fold it in here
                                early += outproj_fillers(1)
                        if not last:
                            issue_loads(1)
                            fillers += proj_groups(0)
                    # at the rep boundary attn(3) still reads kT/v128 rows
                    # 0:512 in its early units (both pairs); the next rep's
                    # proj(0) overwrites them, so its fillers start only after
                    # unit 20 (p=1, kt>=4)
                    min_u = 20 if (qb == 3 and not last) else 0
                    attn_block(qb, fillers, early_fillers=early,
                               filler_min_unit=min_u,
                               do_a2a=phases in ('a2a', 'full'))
                if full and last:
                    for g in outproj_fillers(2):
                        g()
                    for g in outproj_fillers(3):
                        g()
    nc.finalize()
    return nc


_CACHE = {}


def _prep_inputs(query, key, value, Wq, bq, Wk, bk, Wv, bv, Wo, bo, gamma, beta):
    """Host-side shard + transpose + cast. Returns per-core in_maps."""
    q32 = np.asarray(query, np.float32)
    qT = [np.ascontiguousarray(q32[b].T).astype(BF16) for b in range(B)]
    kTt = [np.ascontiguousarray(np.asarray(key, np.float32)[b].T).astype(BF16) for b in range(B)]
    vTt = [np.ascontiguousarray(np.asarray(value, np.float32)[b].T).astype(BF16) for b in range(B)]
    Wqb = np.asarray(Wq, np.float32).astype(BF16)
    Wkb = np.asarray(Wk, np.float32).astype(BF16)
    Wvb = np.asarray(Wv, np.float32).astype(BF16)
    Wob = np.ascontiguousarray(np.asarray(Wo, np.float32)).astype(BF16)
    bo32 = np.asarray(bo, np.float32)
    in_maps = []
    for c in range(NCORES):
        b, g = divmod(c, 4)
        sl = slice(HL * DH * g, HL * DH * (g + 1))
        qres = np.concatenate(
            [q32[b_, 512 * qb + 64 * c: 512 * qb + 64 * c + 64] + bo32
             for qb in range(4) for b_ in range(B)], axis=0)
        in_maps.append({
            "qT": qT[b], "kT": kTt[b], "vT": vTt[b],
            "wq": np.ascontiguousarray(Wqb[:, sl]),
            "wk": np.ascontiguousarray(Wkb[:, sl]),
            "wv": np.ascontiguousarray(Wvb[:, sl]),
            "wo": Wob,
            "bq": np.ascontiguousarray(np.asarray(bq, np.float32)[sl]).reshape(HL * DH, 1),
            "bk": np.ascontiguousarray(np.asarray(bk, np.float32)[sl]).reshape(HL * DH, 1),
            "bv": np.ascontiguousarray(np.asarray(bv, np.float32)[sl].reshape(HL, DH).T),
            "qresbo": np.ascontiguousarray(qres, np.float32),
            "gamma": np.asarray(gamma, np.float32).reshape(1, D),
            "beta": np.asarray(beta, np.float32).reshape(1, D),
        })
    return in_maps


def _assemble(results):
    out = np.empty((B, L, D), np.float32)
    for c in range(NCORES):
        yc = results[c]["y"]
        for b_ in range(B):
            for qb in range(4):
                out[b_, 512 * qb + 64 * c: 512 * qb + 64 * c + 64] = \
                    yc[128 * qb + 64 * b_: 128 * qb + 64 * b_ + 64]
    return out


def kernel(**inputs) -> np.ndarray:
    from concourse.bass_utils import run_bass_kernel_spmd
    in_maps = _prep_inputs(
        inputs["query"], inputs["key"], inputs["value"],
        inputs["Wq"], inputs["bq"], inputs["Wk"], inputs["bk"],
        inputs["Wv"], inputs["bv"], inputs["Wo"], inputs["bo"],
        inputs["gamma"], inputs["beta"])
    wb = any(np.any(np.asarray(inputs[k]) != 0) for k in ("bq", "bk", "bv"))
    lt = (np.all(np.asarray(inputs["gamma"]) == 1.0)
          and np.all(np.asarray(inputs["beta"]) == 0.0))
    key = ("nc", wb, lt)
    if key not in _CACHE:
        _CACHE[key] = build_nc(with_biases=wb, ln_trivial=bool(lt))
    _CACHE["nc"] = _CACHE[key]
    res = run_bass_kernel_spmd(_CACHE[key], in_maps, core_ids=list(range(NCORES)))
    return _assemble(res.results)


if __name__ == "__main__":
    # quick shape check of the program build
    nc = build_nc()
    n_inst = sum(len(bb.instructions) for f in nc.m.functions for bb in f.blocks)
    print("built ok, instructions:", n_inst)
